# revision 1
# baseline (speedup 1.0000x reference)
"""Self-contained Trainium2 Bass kernel for the ARMA GNN problem
(nn_ARMA_49297634623854).

kernel(**inputs) takes the FULL unsharded inputs (x, edge_index, edge_attr,
batch, Wi1, Wr1, b1, Wi2, Wr2, b2, mW1, mb1, mW2, mb2) as numpy arrays,
shards node-contiguously across 8 NeuronCores, runs an SPMD Bass kernel
(indirect-DMA edge gather + one-hot-matmul scatter + AllGather/AllReduce
collectives), and returns the full [512, 2] float32 output.
"""

# ======================= walrus wait-splitting patches =======================
import concourse.mybir as mybir
import concourse.tile as tile
from concourse.vector_clock import ScopedClock, VectorClock

_nop_counter = [0]


def _make_wait_nop(engine, wait):
    _nop_counter[0] += 1
    return mybir.InstNoOp(
        name=f"SplitWait-{_nop_counter[0]}",
        engine=engine,
        ins=[],
        outs=[],
        sync_info=mybir.SyncInfo(on_wait=[wait], on_update=[]),
        bass_nofuse=True,
    )


def _split_multi_waits(insts):
    out = []
    n_split = 0
    for inst in insts:
        si = inst.sync_info
        if si is not None and len(si.on_wait) > 1:
            waits = list(si.on_wait)
            for w in waits[:-1]:
                out.append(_make_wait_nop(inst.engine, w))
            inst.sync_info = mybir.SyncInfo(
                on_wait=[waits[-1]], on_update=list(si.on_update)
            )
            n_split += 1
        out.append(inst)
    return out, n_split


_orig_lower = tile.TileContext._lower_ordered_insts


def _patched_lower(self, postordered_blocks):
    total = 0
    for bbname in list(postordered_blocks.keys()):
        newlist, n = _split_multi_waits(postordered_blocks[bbname])
        postordered_blocks[bbname] = newlist
        total += n
    return _orig_lower(self, postordered_blocks)


def _patched_drain_and_barrier(self, tick_clock, wait_clock):
    gc = tick_clock.global_clock
    nprocs = len(gc)
    for p in range(nprocs):
        t = gc[p]
        if t <= 0:
            continue
        vec = [0] * nprocs
        vec[p] = t
        nop_inst = self.nc.sync.nop(nofuse=True)
        wait_clock.add_sem_waits(nop_inst.ins, ScopedClock({None: VectorClock(vec)}))
    self.nc.sync.drain()
    self.nc.all_engine_barrier()
    assert self.sems is not None
    popped = self.nc._tile_sem_poison_stack.pop()
    assert popped is self._sem_poison
    self.nc.clear_and_free_semaphores(list(self.sems.allocated().values()))
    self.nc.all_engine_barrier()


def install():
    tile.TileContext._lower_ordered_insts = _patched_lower
    tile.TileContext._drain_and_barrier = _patched_drain_and_barrier


# ======================= SPMD runner =======================
import time

import jax
import numpy as np
from jax.sharding import Mesh, NamedSharding, PartitionSpec
from jax.experimental.shard_map import shard_map

import concourse.bass as bass
import concourse.mybir as mybir
from concourse import bass2jax
from concourse.bass2jax import (
    _bass_exec_p,
    fast_dispatch_compile,
    install_neuronx_cc_hook,
    partition_id_tensor,
)


class SpmdKernel:
    def __init__(self, nc: bass.Bass, n_cores: int = 8):
        install_neuronx_cc_hook()
        self.nc = nc
        self.n_cores = n_cores
        in_names: list[str] = []
        out_names: list[str] = []
        out_avals: list[jax.core.ShapedArray] = []
        partition_name = (
            nc.partition_id_tensor.name if nc.partition_id_tensor else None
        )
        for alloc in nc.m.functions[0].allocations:
            if not isinstance(alloc, mybir.MemoryLocationSet):
                continue
            name = alloc.memorylocations[0].name
            if alloc.kind == "ExternalInput":
                if name != partition_name:
                    in_names.append(name)
            elif alloc.kind == "ExternalOutput":
                shape = tuple(alloc.tensor_shape)
                dtype = mybir.dt.np(alloc.dtype)
                out_names.append(name)
                out_avals.append(jax.core.ShapedArray(shape, dtype))
        self.n_params = len(in_names)
        self.out_names = out_names
        self.out_avals = out_avals
        self.in_names = in_names[:]
        all_in_names = in_names + out_names
        if partition_name is not None:
            all_in_names.append(partition_name)

        def _body(*args):
            operands = list(args)
            if partition_name is not None:
                operands.append(partition_id_tensor())
            outs = _bass_exec_p.bind(
                *operands,
                out_avals=tuple(out_avals),
                in_names=tuple(all_in_names),
                out_names=tuple(out_names),
                lowering_input_output_aliases=(),
                sim_require_finite=True,
                sim_require_nnan=True,
                nc=nc,
            )
            return tuple(outs)

        devices = jax.devices()[: n_cores]
        assert len(devices) == n_cores
        self.mesh = Mesh(np.asarray(devices), ("core",))
        n_out = len(out_names)
        in_specs = (PartitionSpec("core"),) * (self.n_params + n_out)
        out_specs = (PartitionSpec("core"),) * n_out
        self._sharded = shard_map(
            _body,
            mesh=self.mesh,
            in_specs=in_specs,
            out_specs=out_specs,
            check_rep=False,
        )
        self.fn = jax.jit(self._sharded, keep_unused=True)
        self._compiled = None
        self.sharding = NamedSharding(self.mesh, PartitionSpec("core"))

    def compile_fast(self, concat_in, zeros):
        """AOT compile with fast dispatch (no effects)."""
        self._compiled = fast_dispatch_compile(
            lambda: jax.jit(self._sharded, keep_unused=True)
            .lower(*concat_in, *zeros)
            .compile()
        )
        return self._compiled

    def put_inputs(self, in_maps: list[dict[str, np.ndarray]]):
        """in_maps: one dict per core. Returns list of device arrays (concat
        along axis 0) in in_names order, plus zero output buffers."""
        concat_in = []
        for name in self.in_names:
            arrs = [np.asarray(in_maps[c][name]) for c in range(self.n_cores)]
            concat_in.append(
                jax.device_put(np.concatenate(arrs, axis=0), self.sharding)
            )
        zeros = []
        for av in self.out_avals:
            z = np.zeros((self.n_cores * av.shape[0], *av.shape[1:]), av.dtype)
            zeros.append(jax.device_put(z, self.sharding))
        return concat_in, zeros

    def __call__(self, concat_in, zeros):
        f = self._compiled or self.fn
        outs = f(*concat_in, *zeros)
        return outs

    def run_np(self, concat_in, zeros):
        f = self._compiled or self.fn
        outs = f(*concat_in, *zeros)
        res = []
        for c in range(self.n_cores):
            res.append(
                {
                    name: np.asarray(outs[i]).reshape(
                        self.n_cores, *self.out_avals[i].shape
                    )[c]
                    for i, name in enumerate(self.out_names)
                }
            )
        return res

    def time_it(self, concat_in, zeros, reps=20, warmup=3):
        f = self._compiled or self.fn
        for _ in range(warmup):
            jax.block_until_ready(f(*concat_in, *zeros))
        ts = []
        for _ in range(reps):
            t0 = time.perf_counter()
            jax.block_until_ready(f(*concat_in, *zeros))
            ts.append(time.perf_counter() - t0)
        return min(ts), sorted(ts)[len(ts) // 2]


# ======================= GNN kernel builder =======================
import math
import numpy as np

import concourse.bass as bass
import concourse.mybir as mybir
import concourse.tile as tile
from concourse.tile_rust import add_dep_helper

F32 = mybir.dt.float32
BF16 = mybir.dt.bfloat16
I32 = mybir.dt.int32
AF = mybir.ActivationFunctionType
OP = mybir.AluOpType
P = 128


def wrap128(arr):
    """[C*128,...] -> [128, C] layout: out[p, c] = arr[c*128+p]."""
    C = arr.shape[0] // 128
    return np.ascontiguousarray(arr.reshape(C, 128).T)


def schedule(C, npc):
    """Fixed chunk->node-window schedule: s_c, window [s_c, s_c+128)."""
    if C == 1:
        return np.zeros(1, np.int64)
    delta = (npc - 128) / (C - 1)
    return np.minimum(np.floor(np.arange(C) * delta).astype(np.int64), npc - 128)


def assign_chunks(local_col_sorted, C, sched):
    """Greedy: edges (sorted by local col) -> chunk slots. Returns slot index
    per edge or None if infeasible."""
    fill = np.zeros(C, np.int32)
    out = np.empty(len(local_col_sorted), np.int64)
    c_lo = 0
    for i, v in enumerate(local_col_sorted):
        c = c_lo
        # first chunk whose window contains v and has space
        while c < C and (fill[c] >= 128 or sched[c] + 128 <= v):
            if fill[c] >= 128 and sched[c] + 128 <= (v + 1):
                c_lo = max(c_lo, c + 1)
            c += 1
        if c >= C or sched[c] > v:
            return None
        out[i] = c * 128 + fill[c]
        fill[c] += 1
    return out


def preprocess(edge_index, edge_attr, batch, n, ncores, g):
    """Build per-core tables. Returns dict with C, sched, per-core arrays."""
    npc = n // ncores
    row = np.asarray(edge_index[0], np.int64)
    col = np.asarray(edge_index[1], np.int64)
    w = np.asarray(edge_attr, np.float32).reshape(-1)
    batch = np.asarray(batch, np.int64)
    e = len(row)

    core_of = col // npc
    per_core = []
    maxe = 0
    for c in range(ncores):
        m = core_of == c
        r_c, c_c, w_c = row[m], col[m] - c * npc, w[m]
        o = np.argsort(c_c, kind="stable")
        per_core.append((r_c[o], c_c[o], w_c[o]))
        maxe = max(maxe, len(r_c))

    C = int(math.ceil(maxe / 128 * 1.12))
    while True:
        sched = schedule(C, npc)
        slots = []
        ok = True
        for r_c, c_c, w_c in per_core:
            s = assign_chunks(c_c, C, sched)
            if s is None:
                ok = False
                break
            slots.append(s)
        if ok:
            break
        C = int(C * 1.06) + 1

    cores = []
    for (r_c, c_c, w_c), s in zip(per_core, slots):
        rowidx = np.zeros(C * 128, np.int32)
        colloc = np.zeros(C * 128, np.float32)
        wvals = np.zeros(C * 128, np.float32)
        rowidx[s] = r_c
        colloc[s] = c_c - sched[s // 128]
        wvals[s] = w_c
        # within each chunk, sort by source row for HBM gather locality
        r3 = rowidx.reshape(C, 128)
        c3 = colloc.reshape(C, 128)
        w3 = wvals.reshape(C, 128)
        o = np.argsort(r3, axis=1, kind="stable")
        r3 = np.take_along_axis(r3, o, axis=1)
        c3 = np.take_along_axis(c3, o, axis=1)
        w3 = np.take_along_axis(w3, o, axis=1)
        cores.append(
            dict(
                rowidx=wrap128(r3.reshape(-1)),
                colloc=wrap128(c3.reshape(-1)).astype(np.float32),
                wvals=wrap128(w3.reshape(-1)).astype(np.float32),
            )
        )

    # pooling tables
    nchunks = (npc + 127) // 128
    npc_pad = nchunks * 128
    gblocks = (g + 127) // 128
    for c in range(ncores):
        bl = np.full(npc_pad, -1.0, np.float32)
        bl[:npc] = batch[c * npc:(c + 1) * npc].astype(np.float32)
        cols = []
        for b in range(gblocks):
            blb = np.where(bl >= 0, bl - 128.0 * b, -1.0)
            cols.append(wrap128(blb))
        cores[c]["batchloc"] = np.concatenate(cols, axis=1)  # [128, nchunks*gblocks]

    return dict(C=C, sched=sched, cores=cores, npc=npc, nchunks=nchunks,
                npc_pad=npc_pad, gblocks=gblocks)


def to_bf16(a):
    import jax.numpy as jnp
    return np.asarray(jnp.asarray(np.asarray(a, np.float32), dtype=jnp.bfloat16))


def build_nc(C, sched, n, ncores, g, ncls, nchunks, npc, npc_pad, gblocks, stop_after=None):
    F = 128
    nc = bass.Bass()

    # ---------------- parameters ----------------
    xT = nc.declare_dram_parameter("xT", [P, npc_pad], BF16, isOutput=False)
    rowidx = nc.declare_dram_parameter("rowidx", [P, C], I32, isOutput=False)
    colloc = nc.declare_dram_parameter("colloc", [P, C], F32, isOutput=False)
    wvals = nc.declare_dram_parameter("wvals", [P, C], F32, isOutput=False)
    batchloc = nc.declare_dram_parameter("batchloc", [P, nchunks * gblocks], F32, isOutput=False)
    iota_p = nc.declare_dram_parameter("iota128", [P, P], F32, isOutput=False)
    ident_p = nc.declare_dram_parameter("ident128", [P, P], F32, isOutput=False)
    wi = [nc.declare_dram_parameter(f"wi{l}", [F, F], BF16, isOutput=False) for l in (1, 2)]
    wr = [nc.declare_dram_parameter(f"wr{l}", [F, F], BF16, isOutput=False) for l in (1, 2)]
    bb = [nc.declare_dram_parameter(f"b{l}", [F, 1], F32, isOutput=False) for l in (1, 2)]
    mw1 = nc.declare_dram_parameter("mw1", [F, 2 * F], F32, isOutput=False)
    mb1 = nc.declare_dram_parameter("mb1", [F, 2], F32, isOutput=False)
    mw2 = nc.declare_dram_parameter("mw2", [P, 2, ncls], F32, isOutput=False)
    mb2 = nc.declare_dram_parameter("mb2", [P, ncls], F32, isOutput=False)
    out = nc.declare_dram_parameter("out", [g, ncls], F32, isOutput=True)

    # ---------------- internal DRAM ----------------
    m_local = nc.dram_tensor("m_local", [npc, F], BF16)
    m_full = nc.dram_tensor("m_full", [n, F], BF16, addr_space="Shared")
    deg_dram = nc.dram_tensor("deg_dram", [1, npc_pad], F32)
    dinv_dram = nc.dram_tensor("dinv_dram", [1, npc_pad], F32)
    pool_part = nc.dram_tensor("pool_part", [gblocks * P, F + 1], F32)
    pool_red = nc.dram_tensor("pool_red", [gblocks * P, F + 1], F32, addr_space="Shared")

    cc_groups = [list(range(ncores))]

    import contextlib
    es = contextlib.ExitStack()
    # ---------------- persistent SBUF state ----------------
    hT = es.enter_context(nc.sbuf_tensor("hT", [P, npc_pad], BF16))
    aggT = es.enter_context(nc.sbuf_tensor("aggT", [P, npc], F32))
    dinv_bc = es.enter_context(nc.sbuf_tensor("dinv_bc", [P, npc_pad], F32))
    dinv_nm = es.enter_context(nc.sbuf_tensor("dinv_nm", [P, nchunks], F32))
    iota_t = es.enter_context(nc.sbuf_tensor("iota_t", [P, P], F32))
    ident_t = es.enter_context(nc.sbuf_tensor("ident_t", [P, P], F32))
    ident_bf = es.enter_context(nc.sbuf_tensor("ident_bf", [P, P], BF16))
    ones_t = es.enter_context(nc.sbuf_tensor("ones_t", [P, 1], BF16))
    rowidx_t = es.enter_context(nc.sbuf_tensor("rowidx_t", [P, C], I32))
    colloc_t = es.enter_context(nc.sbuf_tensor("colloc_t", [P, C], F32))
    wvals_t = es.enter_context(nc.sbuf_tensor("wvals_t", [P, C], F32))
    batchloc_t = es.enter_context(nc.sbuf_tensor("batchloc_t", [P, nchunks * gblocks], F32))
    wi_t = [es.enter_context(nc.sbuf_tensor(f"wi_t{l}", [F, F], BF16)) for l in range(2)]
    wr_t = [es.enter_context(nc.sbuf_tensor(f"wr_t{l}", [F, F], BF16)) for l in range(2)]
    b_t = [es.enter_context(nc.sbuf_tensor(f"b_t{l}", [F, 1], F32)) for l in range(2)]
    cc_sem = es.enter_context(nc.semaphore("cc_sem"))

    def gen_ohw(pool, c, tag, after=None):
        oh = pool.tile([P, P], BF16, tag=tag, name=f"oh_{tag}_{c}")
        inst = nc.vector.tensor_scalar(
            out=oh[:], in0=iota_t[:],
            scalar1=colloc_t[:, c:c + 1], scalar2=wvals_t[:, c:c + 1],
            op0=OP.is_equal, op1=OP.mult,
        )
        if after is not None:
            add_dep_helper(inst.ins, after.ins, reason="pace oh behind gather")
        return oh

    def m_pass(l, sm, psm):
        """m~ = dinv * (h @ Wi) -> m_local DRAM (node-major bf16)."""
        for ci in range(nchunks):
            s0, s1 = ci * P, (ci + 1) * P
            rows = min(npc - s0, P)
            mp = psm.tile([P, F], F32, tag="ps", name=f"mp{l}_{ci}")
            nc.tensor.matmul(out=mp[:], lhsT=hT[:, s0:s1], rhs=wi_t[l][:],
                             start=True, stop=True)
            ms = sm.tile([P, F], BF16, tag="ms", name=f"ms{l}_{ci}")
            nc.vector.tensor_scalar(out=ms[:], in0=mp[:],
                                    scalar1=dinv_nm[:, ci:ci + 1],
                                    scalar2=None, op0=OP.mult)
            nc.sync.dma_start(out=m_local[s0:s0 + rows, :], in_=ms[:rows, :])

    # ================= phase A: loads, deg, dinv, m1 =================
    with tile.TileContext(nc) as tc:
        with (
            tc.tile_pool(name="sm", bufs=4) as sm,
            tc.tile_pool(name="big", bufs=1) as big,
            tc.tile_pool(name="ohp", bufs=8) as ohp,
            tc.tile_pool(name="psm", bufs=4, space="PSUM") as psm,
        ):
            nc.sync.dma_start(out=iota_t[:], in_=iota_p[:])
            nc.sync.dma_start(out=ident_t[:], in_=ident_p[:])
            nc.vector.tensor_copy(ident_bf[:], ident_t[:])
            nc.vector.memset(ones_t[:], 1.0)
            nc.sync.dma_start(out=rowidx_t[:], in_=rowidx[:])
            nc.sync.dma_start(out=colloc_t[:], in_=colloc[:])
            nc.sync.dma_start(out=wvals_t[:], in_=wvals[:])
            nc.sync.dma_start(out=batchloc_t[:], in_=batchloc[:])
            for l in range(2):
                nc.sync.dma_start(out=wi_t[l][:], in_=wi[l][:])
                nc.sync.dma_start(out=wr_t[l][:], in_=wr[l][:])
                nc.sync.dma_start(out=b_t[l][:], in_=bb[l][:])
            nc.sync.dma_start(out=hT[:], in_=xT[:])

            degT = big.tile([1, npc_pad], F32)
            nc.vector.memset(degT[:], 0.0)
            for c in range(C):
                oh = gen_ohw(ohp, c, "ohd")
                dps = psm.tile([1, P], F32, tag="ps", name=f"dps{c}")
                nc.tensor.matmul(out=dps[:], lhsT=ones_t[:], rhs=oh[:],
                                 start=True, stop=True)
                s = int(sched[c])
                nc.vector.tensor_add(degT[0:1, s:s + P], degT[0:1, s:s + P], dps[:])
            d_w = nc.sync.dma_start(out=deg_dram[:], in_=degT[:])
            deg_nm_v = deg_dram.rearrange("one (c p) -> p (one c)", p=P)
            deg_nm = sm.tile([P, nchunks], F32, tag="deg_nm")
            d_r = nc.sync.dma_start(out=deg_nm[:], in_=deg_nm_v)
            add_dep_helper(d_r.ins, d_w.ins, reason="deg roundtrip")
            t0 = sm.tile([P, nchunks], F32, tag="dt0")
            nc.vector.tensor_scalar(out=t0[:], in0=deg_nm[:], scalar1=1e-12,
                                    scalar2=None, op0=OP.max)
            t1 = sm.tile([P, nchunks], F32, tag="dt1")
            nc.scalar.activation(out=t1[:], in_=t0[:], func=AF.Sqrt)
            t2 = sm.tile([P, nchunks], F32, tag="dt2")
            nc.vector.reciprocal(t2[:], t1[:])
            t3 = sm.tile([P, nchunks], F32, tag="dt3")
            nc.vector.tensor_scalar(out=t3[:], in0=deg_nm[:], scalar1=0.0,
                                    scalar2=None, op0=OP.not_equal)
            nc.vector.tensor_tensor(out=dinv_nm[:], in0=t2[:], in1=t3[:], op=OP.mult)
            with nc.allow_non_contiguous_dma(reason="dinv scatter-store [P,nchunks]"):
                di_w = nc.sync.dma_start(
                    out=dinv_dram.rearrange("one (c p) -> p (one c)", p=P),
                    in_=dinv_nm[:])
            di_r = nc.sync.dma_start(
                out=dinv_bc[:], in_=dinv_dram[0:1, :].to_broadcast([P, npc_pad]))
            add_dep_helper(di_r.ins, di_w.ins, reason="dinv roundtrip")

            m_pass(0, sm, psm)

    def finish_early():
        with tile.TileContext(nc) as tc:
            with tc.tile_pool(name="fe", bufs=1) as fe:
                z = fe.tile([min(P, g), gblocks, ncls], F32)
                nc.vector.memset(z[:], 0.0)
                nc.sync.dma_start(
                    out=out.rearrange("(b p) c -> p b c", p=min(P, g)), in_=z[:])
        es.close()
        return nc

    if stop_after == "A":
        return finish_early()

    # ================= conv layers =================
    for l in range(2):
        nc.gpsimd.collective_compute(
            "AllGather", OP.bypass, replica_groups=cc_groups,
            ins=[m_local[:]], outs=[m_full[:]]).then_inc(cc_sem, 1)
        nc.gpsimd.wait_ge(cc_sem, l + 1)
        nc.sync.wait_ge(cc_sem, l + 1)

        with tile.TileContext(nc) as tc:
            with (
                tc.tile_pool(name="sm", bufs=4) as sm,
                tc.tile_pool(name="msgp", bufs=8) as msgp,
                tc.tile_pool(name="ohp", bufs=8) as ohp,
                tc.tile_pool(name="psm", bufs=4, space="PSUM") as psm,
            ):
                nc.vector.memset(aggT[:], 0.0)
                for c in range(C):
                    msg = msgp.tile([P, F], BF16, tag="msg", name=f"msg{l}_{c}")
                    gi = nc.gpsimd.indirect_dma_start(
                        out=msg[:], out_offset=None, in_=m_full[:],
                        in_offset=bass.IndirectOffsetOnAxis(
                            ap=rowidx_t[:, c:c + 1], axis=0))
                    oh = gen_ohw(ohp, c, "oh", after=gi)
                    sc = psm.tile([F, P], F32, tag="ps", name=f"sc{l}_{c}")
                    nc.tensor.matmul(out=sc[:], lhsT=msg[:], rhs=oh[:],
                                     start=True, stop=True)
                    s = int(sched[c])
                    nc.vector.tensor_add(aggT[:, s:s + P], aggT[:, s:s + P], sc[:])

                # post: h' = relu(aggT * dinv + h @ Wr + b)
                for ci in range(nchunks):
                    s0 = ci * P
                    s1 = s0 + P
                    e0 = min(npc, s1)
                    w_ = e0 - s0
                    hp = psm.tile([F, P], F32, tag="ps", name=f"hp{l}_{ci}")
                    nc.tensor.matmul(out=hp[:], lhsT=wr_t[l][:], rhs=hT[:, s0:s1],
                                     start=True, stop=True)
                    u0 = sm.tile([P, P], F32, tag="u0", name=f"u0_{l}_{ci}")
                    nc.vector.tensor_tensor(out=u0[:, :w_], in0=aggT[:, s0:e0],
                                            in1=dinv_bc[:, s0:e0], op=OP.mult)
                    u1 = sm.tile([P, P], F32, tag="u1", name=f"u1_{l}_{ci}")
                    nc.vector.tensor_tensor(out=u1[:, :w_], in0=u0[:, :w_],
                                            in1=hp[:, :w_], op=OP.add)
                    if w_ < P:
                        nc.vector.memset(hT[:, s0 + w_:s1], 0.0)
                    nc.vector.tensor_scalar(out=hT[:, s0:e0], in0=u1[:, :w_],
                                            scalar1=b_t[l][:, 0:1], scalar2=0.0,
                                            op0=OP.add, op1=OP.max)
                if l == 0:
                    m_pass(1, sm, psm)
        if stop_after == f"conv{l}ag":
            break
        if stop_after == f"conv{l}":
            return finish_early()

    if stop_after in ("conv0ag", "conv1ag"):
        return finish_early()

    # ================= pooling + MLP =================
    with tile.TileContext(nc) as tc:
        with (
            tc.tile_pool(name="sm", bufs=4) as sm,
            tc.tile_pool(name="ohp", bufs=8) as ohp,
            tc.tile_pool(name="psm", bufs=4, space="PSUM") as psm,
            tc.tile_pool(name="psPool", bufs=1, space="PSUM") as psPool,
        ):
            pool_ps = [psPool.tile([P, F + 1], F32, tag=f"pp{b}", name=f"pool_ps{b}")
                       for b in range(gblocks)]
            for ci in range(nchunks):
                s0 = ci * P
                tp = psm.tile([P, F], BF16, tag="ps", name=f"tp{ci}")
                nc.tensor.transpose(out=tp[:], in_=hT[:, s0:s0 + P], identity=ident_bf[:])
                nxr = sm.tile([P, F + 1], BF16, tag="nxr", name=f"nxr{ci}")
                nc.scalar.activation(out=nxr[:, :F], in_=tp[:], func=AF.Copy)
                nc.vector.memset(nxr[:, F:F + 1], 1.0)
                for b in range(gblocks):
                    ohg = ohp.tile([P, P], BF16, tag="ohg", name=f"ohg{ci}_{b}")
                    nc.vector.tensor_scalar(
                        out=ohg[:], in0=iota_t[:],
                        scalar1=batchloc_t[:, b * nchunks + ci:b * nchunks + ci + 1],
                        scalar2=None, op0=OP.is_equal)
                    nc.tensor.matmul(out=pool_ps[b][:], lhsT=ohg[:], rhs=nxr[:],
                                     start=(ci == 0), stop=(ci == nchunks - 1))
            for b in range(gblocks):
                pps = sm.tile([P, F + 1], F32, tag="pps", name=f"pps{b}")
                nc.scalar.activation(out=pps[:], in_=pool_ps[b][:], func=AF.Copy)
                nc.sync.dma_start(out=pool_part[b * P:(b + 1) * P, :], in_=pps[:])

    if stop_after == "pool":
        return finish_early()

    nc.gpsimd.collective_compute(
        "AllReduce", OP.add, replica_groups=cc_groups,
        ins=[pool_part[:]], outs=[pool_red[:]]).then_inc(cc_sem, 1)
    nc.sync.wait_ge(cc_sem, 3)

    if stop_after == "ar":
        return finish_early()

    with tile.TileContext(nc) as tc:
        with (
            tc.tile_pool(name="sm", bufs=4) as sm,
            tc.tile_pool(name="one", bufs=1) as one,
            tc.tile_pool(name="psm", bufs=4, space="PSUM") as psm,
        ):
            meanT = one.tile([F, gblocks * P], F32)
            for b in range(gblocks):
                pr = sm.tile([P, F + 1], F32, tag="pr", name=f"pr{b}")
                nc.sync.dma_start(out=pr[:], in_=pool_red[b * P:(b + 1) * P, :])
                cnt = sm.tile([P, 1], F32, tag="cnt", name=f"cnt{b}")
                nc.vector.tensor_scalar(out=cnt[:], in0=pr[:, F:F + 1], scalar1=1.0,
                                        scalar2=None, op0=OP.max)
                rec = sm.tile([P, 1], F32, tag="rec", name=f"rec{b}")
                nc.vector.reciprocal(rec[:], cnt[:])
                mg = sm.tile([P, F], F32, tag="mg", name=f"mg{b}")
                nc.vector.tensor_scalar(out=mg[:], in0=pr[:, :F], scalar1=rec[:, 0:1],
                                        scalar2=None, op0=OP.mult)
                mt = psm.tile([F, P], F32, tag="ps", name=f"mt{b}")
                nc.tensor.transpose(out=mt[:], in_=mg[:], identity=ident_t[:])
                nc.scalar.activation(out=meanT[:, b * P:(b + 1) * P], in_=mt[:],
                                     func=AF.Copy)
            if stop_after == "mean":
                z = sm.tile([min(P, g), gblocks, ncls], F32, tag="z")
                nc.vector.memset(z[:], 0.0)
                nc.sync.dma_start(
                    out=out.rearrange("(b p) c -> p b c", p=min(P, g)), in_=z[:])
                es.close()
                return nc
            mw1_t = one.tile([F, 2 * F], F32)
            nc.sync.dma_start(out=mw1_t[:], in_=mw1[:])
            mb1_t = one.tile([F, 2], F32)
            nc.sync.dma_start(out=mb1_t[:], in_=mb1[:])
            mw2_t = one.tile([P, 2, ncls], F32)
            nc.sync.dma_start(out=mw2_t[:], in_=mw2[:])
            mb2_t = one.tile([P, ncls], F32)
            nc.sync.dma_start(out=mb2_t[:], in_=mb2[:])
            hidT = one.tile([F, 2, gblocks * P], F32)
            for hc in range(2):
                hps = psm.tile([F, gblocks * P], F32, tag="hps", name=f"hps{hc}")
                nc.tensor.matmul(out=hps[:], lhsT=mw1_t[:, hc * F:(hc + 1) * F],
                                 rhs=meanT[:], start=True, stop=True)
                nc.scalar.activation(out=hidT[:, hc, :], in_=hps[:], func=AF.Relu,
                                     bias=mb1_t[:, hc:hc + 1], scale=1.0)
            p_out = min(P, g)
            outsb = one.tile([P, gblocks, ncls], F32)
            for gc in range(gblocks):
                ops_ = psm.tile([P, ncls], F32, tag="ps", name=f"ops{gc}")
                for hc in range(2):
                    nc.tensor.matmul(out=ops_[:], lhsT=hidT[:, hc, gc * P:(gc + 1) * P],
                                     rhs=mw2_t[:, hc, :],
                                     start=(hc == 0), stop=(hc == 1))
                nc.vector.tensor_tensor(out=outsb[:, gc, :], in0=ops_[:],
                                        in1=mb2_t[:], op=OP.add)
            nc.sync.dma_start(
                out=out.rearrange("(b p) c -> p b c", p=p_out),
                in_=outsb[:p_out, :, :])

    es.close()
    return nc


def make_inputs(pre, x, Wi1, Wr1, b1, Wi2, Wr2, b2, mW1, mb1, mW2, mb2,
                n, ncores, g, ncls):
    """Build per-core in_maps."""
    npc = pre["npc"]
    npc_pad = pre["npc_pad"]
    iota = np.tile(np.arange(P, dtype=np.float32)[None, :], (P, 1))
    ident = np.eye(P, dtype=np.float32)
    x = np.asarray(x, np.float32)
    in_maps = []
    mb1w = np.ascontiguousarray(np.asarray(mb1, np.float32).reshape(2, P).T)
    mb2r = np.tile(np.asarray(mb2, np.float32).reshape(1, ncls), (P, 1))
    for c in range(ncores):
        xs = np.zeros((P, npc_pad), np.float32)
        xs[:, :npc] = x[c * npc:(c + 1) * npc, :].T
        m = dict(
            xT=to_bf16(xs),
            rowidx=pre["cores"][c]["rowidx"],
            colloc=pre["cores"][c]["colloc"],
            wvals=pre["cores"][c]["wvals"],
            batchloc=pre["cores"][c]["batchloc"],
            iota128=iota,
            ident128=ident,
            wi1=to_bf16(Wi1), wr1=to_bf16(Wr1),
            wi2=to_bf16(Wi2), wr2=to_bf16(Wr2),
            b1=np.asarray(b1, np.float32).reshape(P, 1),
            b2=np.asarray(b2, np.float32).reshape(P, 1),
            mw1=np.asarray(mW1, np.float32),
            mb1=mb1w,
            mw2=np.ascontiguousarray(
                np.asarray(mW2, np.float32).reshape(2, P, ncls).transpose(1, 0, 2)),
            mb2=mb2r,
        )
        in_maps.append(m)
    return in_maps


# ======================= entry point =======================
N_FULL = 100000
E_FULL = 640000
G_FULL = 512
NCLS_FULL = 2
NCORES = 8

_cache = {}


def kernel(x, edge_index, edge_attr, batch, Wi1, Wr1, b1, Wi2, Wr2, b2,
           mW1, mb1, mW2, mb2):
    install()
    x = np.asarray(x)
    edge_index = np.asarray(edge_index)
    edge_attr = np.asarray(edge_attr)
    batch = np.asarray(batch)
    n, f = x.shape
    g = G_FULL
    ncls = np.asarray(mW2).shape[1]

    pre = preprocess(edge_index, edge_attr, batch, n, NCORES, g)
    key = (n, g, ncls, pre["C"])
    if key not in _cache:
        nc = build_nc(pre["C"], pre["sched"], n, NCORES, g, ncls,
                      pre["nchunks"], pre["npc"], pre["npc_pad"], pre["gblocks"])
        _cache[key] = SpmdKernel(nc)
    k = _cache[key]
    in_maps = make_inputs(pre, x, Wi1, Wr1, b1, Wi2, Wr2, b2,
                          mW1, mb1, mW2, mb2, n, NCORES, g, ncls)
    ci, zz = k.put_inputs(in_maps)
    res = k.run_np(ci, zz)
    return np.ascontiguousarray(res[0]["out"].astype(np.float32))



# revision 10
# speedup vs baseline: 33.2288x; 33.2288x over previous
"""Self-contained Trainium2 Bass kernel for the ARMA GNN problem
(nn_ARMA_49297634623854).

kernel(**inputs) takes the FULL unsharded inputs (x, edge_index, edge_attr,
batch, Wi1, Wr1, b1, Wi2, Wr2, b2, mW1, mb1, mW2, mb2) as numpy arrays,
shards node-contiguously across 8 NeuronCores, runs an SPMD Bass kernel
(batched dma_gather edge gather + one-hot-matmul scatter fused in PSUM +
AllGather/AllReduce collectives), and returns the full [512, 2] float32
output.

Design notes:
- gcn_norm (deg/dinv) is computed on the host and baked into per-edge
  weights, so the device never computes degrees or rescales messages.
- Edge messages m = h @ Wi are AllGathered (bf16), then gathered per-edge
  with batched dma_gather (int16 indices -> 4 source-range groups of 25000
  rows each).
- Edges are grouped by (source-range group, 128-wide dest window); for each
  window all scatter matmuls plus the Wr-path matmul accumulate into one
  PSUM tile, and a single scalar-engine activation (relu + bias) writes the
  new hT window. No aggregation buffer, no vector adds.
"""

# ======================= walrus wait-splitting patches =======================
import concourse.mybir as mybir
import concourse.tile as tile
from concourse.vector_clock import ScopedClock, VectorClock

_nop_counter = [0]


def _make_wait_nop(engine, wait):
    _nop_counter[0] += 1
    return mybir.InstNoOp(
        name=f"SplitWait-{_nop_counter[0]}",
        engine=engine,
        ins=[],
        outs=[],
        sync_info=mybir.SyncInfo(on_wait=[wait], on_update=[]),
        bass_nofuse=True,
    )


def _split_multi_waits(insts):
    out = []
    n_split = 0
    for inst in insts:
        si = inst.sync_info
        if si is not None and len(si.on_wait) > 1:
            waits = list(si.on_wait)
            for w in waits[:-1]:
                out.append(_make_wait_nop(inst.engine, w))
            inst.sync_info = mybir.SyncInfo(
                on_wait=[waits[-1]], on_update=list(si.on_update)
            )
            n_split += 1
        out.append(inst)
    return out, n_split


_orig_lower = tile.TileContext._lower_ordered_insts


def _patched_lower(self, postordered_blocks):
    total = 0
    for bbname in list(postordered_blocks.keys()):
        newlist, n = _split_multi_waits(postordered_blocks[bbname])
        postordered_blocks[bbname] = newlist
        total += n
    return _orig_lower(self, postordered_blocks)


def _patched_drain_and_barrier(self, tick_clock, wait_clock):
    gc = tick_clock.global_clock
    nprocs = len(gc)
    for p in range(nprocs):
        t = gc[p]
        if t <= 0:
            continue
        vec = [0] * nprocs
        vec[p] = t
        nop_inst = self.nc.sync.nop(nofuse=True)
        wait_clock.add_sem_waits(nop_inst.ins, ScopedClock({None: VectorClock(vec)}))
    self.nc.sync.drain()
    self.nc.all_engine_barrier()
    assert self.sems is not None
    popped = self.nc._tile_sem_poison_stack.pop()
    assert popped is self._sem_poison
    self.nc.clear_and_free_semaphores(list(self.sems.allocated().values()))
    self.nc.all_engine_barrier()


def install():
    tile.TileContext._lower_ordered_insts = _patched_lower
    tile.TileContext._drain_and_barrier = _patched_drain_and_barrier


# ======================= SPMD runner =======================
import time

import jax
import numpy as np
from jax.sharding import Mesh, NamedSharding, PartitionSpec
from jax.experimental.shard_map import shard_map

import concourse.bass as bass
import concourse.mybir as mybir
from concourse import bass2jax
from concourse.bass2jax import (
    _bass_exec_p,
    fast_dispatch_compile,
    install_neuronx_cc_hook,
    partition_id_tensor,
)


class SpmdKernel:
    def __init__(self, nc: bass.Bass, n_cores: int = 8):
        install_neuronx_cc_hook()
        self.nc = nc
        self.n_cores = n_cores
        in_names: list[str] = []
        out_names: list[str] = []
        out_avals: list[jax.core.ShapedArray] = []
        partition_name = (
            nc.partition_id_tensor.name if nc.partition_id_tensor else None
        )
        for alloc in nc.m.functions[0].allocations:
            if not isinstance(alloc, mybir.MemoryLocationSet):
                continue
            name = alloc.memorylocations[0].name
            if alloc.kind == "ExternalInput":
                if name != partition_name:
                    in_names.append(name)
            elif alloc.kind == "ExternalOutput":
                shape = tuple(alloc.tensor_shape)
                dtype = mybir.dt.np(alloc.dtype)
                out_names.append(name)
                out_avals.append(jax.core.ShapedArray(shape, dtype))
        self.n_params = len(in_names)
        self.out_names = out_names
        self.out_avals = out_avals
        self.in_names = in_names[:]
        all_in_names = in_names + out_names
        if partition_name is not None:
            all_in_names.append(partition_name)

        def _body(*args):
            operands = list(args)
            if partition_name is not None:
                operands.append(partition_id_tensor())
            outs = _bass_exec_p.bind(
                *operands,
                out_avals=tuple(out_avals),
                in_names=tuple(all_in_names),
                out_names=tuple(out_names),
                lowering_input_output_aliases=(),
                sim_require_finite=True,
                sim_require_nnan=True,
                nc=nc,
            )
            return tuple(outs)

        devices = jax.devices()[: n_cores]
        assert len(devices) == n_cores
        self.mesh = Mesh(np.asarray(devices), ("core",))
        n_out = len(out_names)
        in_specs = (PartitionSpec("core"),) * (self.n_params + n_out)
        out_specs = (PartitionSpec("core"),) * n_out
        self._sharded = shard_map(
            _body,
            mesh=self.mesh,
            in_specs=in_specs,
            out_specs=out_specs,
            check_rep=False,
        )
        self.fn = jax.jit(self._sharded, keep_unused=True)
        self._compiled = None
        self.sharding = NamedSharding(self.mesh, PartitionSpec("core"))

    def compile_fast(self, concat_in, zeros):
        """AOT compile with fast dispatch (no effects)."""
        self._compiled = fast_dispatch_compile(
            lambda: jax.jit(self._sharded, keep_unused=True)
            .lower(*concat_in, *zeros)
            .compile()
        )
        return self._compiled

    def put_inputs(self, in_maps: list[dict[str, np.ndarray]]):
        """in_maps: one dict per core. Returns list of device arrays (concat
        along axis 0) in in_names order, plus zero output buffers."""
        concat_in = []
        for name in self.in_names:
            arrs = [np.asarray(in_maps[c][name]) for c in range(self.n_cores)]
            concat_in.append(
                jax.device_put(np.concatenate(arrs, axis=0), self.sharding)
            )
        zeros = []
        for av in self.out_avals:
            z = np.zeros((self.n_cores * av.shape[0], *av.shape[1:]), av.dtype)
            zeros.append(jax.device_put(z, self.sharding))
        return concat_in, zeros

    def __call__(self, concat_in, zeros):
        f = self._compiled or self.fn
        outs = f(*concat_in, *zeros)
        return outs

    def run_np(self, concat_in, zeros):
        f = self._compiled or self.fn
        outs = f(*concat_in, *zeros)
        res = []
        for c in range(self.n_cores):
            res.append(
                {
                    name: np.asarray(outs[i]).reshape(
                        self.n_cores, *self.out_avals[i].shape
                    )[c]
                    for i, name in enumerate(self.out_names)
                }
            )
        return res

    def time_it(self, concat_in, zeros, reps=20, warmup=3):
        f = self._compiled or self.fn
        for _ in range(warmup):
            jax.block_until_ready(f(*concat_in, *zeros))
        ts = []
        for _ in range(reps):
            t0 = time.perf_counter()
            jax.block_until_ready(f(*concat_in, *zeros))
            ts.append(time.perf_counter() - t0)
        return min(ts), sorted(ts)[len(ts) // 2]


# ======================= GNN kernel builder =======================
import concourse.bass as bass
import concourse.mybir as mybir
import concourse.tile as tile

F32 = mybir.dt.float32
BF16 = mybir.dt.bfloat16
I16 = mybir.dt.int16
AF = mybir.ActivationFunctionType
OP = mybir.AluOpType
P = 128
RANGE = 25000  # int16-safe dma_gather source range
NSB = 4        # gather superblocks per layer per group


def wrap128(arr):
    """[C*128,...] -> [128, C] layout: out[p, c] = arr[c*128+p]."""
    C = arr.shape[0] // 128
    return np.ascontiguousarray(arr.reshape(C, 128).T)


def to_bf16(a):
    import jax.numpy as jnp
    return np.asarray(jnp.asarray(np.asarray(a, np.float32), dtype=jnp.bfloat16))


def preprocess(edge_index, edge_attr, batch, n, ncores, g):
    """Host-side: gcn_norm, per-core edge->chunk tables, pooling tables.

    Chunks are 128 edge slots grouped by (source-range group, dest window).
    Chunk ordering: for sb: for group: for window in sb: chunks -- so each
    (sb, group) is one contiguous dma_gather call span.
    """
    npc = n // ncores
    nw = (npc + 127) // 128
    npc_pad = nw * P
    ngroups = (n + RANGE - 1) // RANGE
    gblocks = (g + P - 1) // P

    row = np.asarray(edge_index[0], np.int64)
    col = np.asarray(edge_index[1], np.int64)
    w_attr = np.asarray(edge_attr, np.float32).reshape(-1)
    batch = np.asarray(batch, np.int64)

    deg = np.bincount(col, weights=w_attr.astype(np.float64), minlength=n)
    dinv = np.where(deg > 0, 1.0 / np.sqrt(np.maximum(deg, 1e-12)), 0.0)
    norm = (dinv[row] * w_attr * dinv[col]).astype(np.float32)

    per_core = []
    cnts = np.zeros((ncores, ngroups, nw), np.int64)
    for c in range(ncores):
        m = (col // npc) == c
        r_c = row[m]
        d_c = col[m] - c * npc
        n_c = norm[m]
        g_c = r_c // RANGE
        w_c = d_c // P
        o = np.lexsort((r_c, w_c, g_c))
        r_c, d_c, n_c, g_c, w_c = r_c[o], d_c[o], n_c[o], g_c[o], w_c[o]
        per_core.append((r_c, d_c, n_c, g_c, w_c))
        for gi in range(ngroups):
            cnts[c, gi] = np.bincount(w_c[g_c == gi], minlength=nw)
    nch = ((cnts.max(axis=0) + P - 1) // P).astype(np.int64)  # [ngroups, nw]

    # window -> superblock (balanced, monotone)
    wsb = (np.arange(nw) * NSB) // nw
    chunk_of = {}
    calls = []  # (sb, g, c0, c1)
    C = 0
    for sb in range(NSB):
        ws = [w for w in range(nw) if wsb[w] == sb]
        for gi in range(ngroups):
            c0 = C
            for w in ws:
                chunk_of[(gi, w)] = C
                C += int(nch[gi, w])
            calls.append((sb, gi, c0, C))
    kbmax = max(c1 - c0 for (_, _, c0, c1) in calls)

    # per-window chunk list: (call_index, local_col) per chunk
    win_chunks = []  # [nw] -> list of (call_idx, local_col)
    call_idx_of = {}
    for i, (sb, gi, c0, c1) in enumerate(calls):
        call_idx_of[(sb, gi)] = i
    for w in range(nw):
        sb = int(wsb[w])
        lst = []
        for gi in range(ngroups):
            k = int(nch[gi, w])
            if k == 0:
                continue
            ci = call_idx_of[(sb, gi)]
            c0 = calls[ci][2]
            base = chunk_of[(gi, w)] - c0
            for j in range(k):
                lst.append((ci, base + j))
        win_chunks.append(lst)

    cores = []
    for c in range(ncores):
        r_c, d_c, n_c, g_c, w_c = per_core[c]
        idx16 = np.zeros(C * P, np.int16)
        colloc = np.zeros(C * P, np.float32)
        wvals = np.zeros(C * P, np.float32)
        key = g_c * nw + w_c
        uniq, starts = np.unique(key, return_index=True)
        ends = np.r_[starts[1:], len(key)]
        for u, s0, s1 in zip(uniq, starts, ends):
            gi, w = divmod(int(u), nw)
            base = chunk_of[(gi, w)] * P
            cnt = int(s1 - s0)
            sl = slice(base, base + cnt)
            idx16[sl] = (r_c[s0:s1] - gi * RANGE).astype(np.int16)
            colloc[sl] = (d_c[s0:s1] - w * P).astype(np.float32)
            wvals[sl] = n_c[s0:s1]
        idx_t = np.tile(np.ascontiguousarray(idx16.reshape(C * 8, 16).T), (8, 1))
        cores.append(
            dict(
                idxs=np.ascontiguousarray(idx_t),
                colloc=wrap128(colloc),
                wvals=wrap128(wvals),
            )
        )

    # pooling: unified (node-chunk ci, graph-block b) pairs
    pair_set = set()
    for c in range(ncores):
        bl = batch[c * npc:(c + 1) * npc]
        for ci in range(nw):
            seg = bl[ci * P: min((ci + 1) * P, npc)]
            for b in np.unique(seg // P):
                pair_set.add((ci, int(b)))
    pairs = sorted(pair_set)
    first_ci = {}
    last_ci = {}
    for (ci, b) in pairs:
        if b not in first_ci:
            first_ci[b] = ci
        last_ci[b] = ci
    pair_flags = []
    for (ci, b) in pairs:
        pair_flags.append((ci, b, ci == first_ci[b], ci == last_ci[b]))

    for c in range(ncores):
        bl = np.full(npc_pad, -1.0, np.float32)
        bl[:npc] = batch[c * npc:(c + 1) * npc].astype(np.float32)
        bt = np.full((P, len(pairs)), -1.0, np.float32)
        for j, (ci, b) in enumerate(pairs):
            seg = bl[ci * P:(ci + 1) * P]
            bt[:, j] = np.where(seg >= 0, seg - P * b, -1.0)
        cores[c]["batchloc"] = np.ascontiguousarray(bt)

    cnt = np.bincount(batch, minlength=g).astype(np.float64)
    cnt_inv = (1.0 / np.maximum(cnt, 1.0)).astype(np.float32)  # [g]
    cntinv_t = np.ascontiguousarray(cnt_inv.reshape(gblocks, P).T)  # [P, gblocks]

    return dict(
        C=C, calls=calls, win_chunks=win_chunks, kbmax=kbmax, nw=nw,
        npc=npc, npc_pad=npc_pad, ngroups=ngroups, gblocks=gblocks,
        pair_flags=pair_flags, npairs=len(pairs), cores=cores,
        cntinv=cntinv_t, wsb=[int(x) for x in wsb],
    )


def build_nc(pre, n, ncores, g, ncls):
    F = 128
    C = pre["C"]
    nw = pre["nw"]
    npc = pre["npc"]
    npc_pad = pre["npc_pad"]
    ngroups = pre["ngroups"]
    gblocks = pre["gblocks"]
    calls = pre["calls"]
    win_chunks = pre["win_chunks"]
    kbmax = pre["kbmax"]
    pair_flags = pre["pair_flags"]
    npairs = pre["npairs"]

    nc = bass.Bass()

    # ---------------- parameters ----------------
    xT = nc.declare_dram_parameter("xT", [P, npc_pad], BF16, isOutput=False)
    idxs_p = nc.declare_dram_parameter("idxs", [P, C * 8], I16, isOutput=False)
    colloc = nc.declare_dram_parameter("colloc", [P, C], F32, isOutput=False)
    wvals = nc.declare_dram_parameter("wvals", [P, C], F32, isOutput=False)
    batchloc = nc.declare_dram_parameter("batchloc", [P, npairs], F32, isOutput=False)
    cntinv = nc.declare_dram_parameter("cntinv", [P, gblocks], F32, isOutput=False)
    iota_p = nc.declare_dram_parameter("iota128", [P, P], F32, isOutput=False)
    ident_p = nc.declare_dram_parameter("ident128", [P, P], F32, isOutput=False)
    wi = [nc.declare_dram_parameter(f"wi{l}", [F, F], BF16, isOutput=False) for l in (1, 2)]
    wr = [nc.declare_dram_parameter(f"wr{l}", [F, F], BF16, isOutput=False) for l in (1, 2)]
    bb = [nc.declare_dram_parameter(f"b{l}", [F, 1], F32, isOutput=False) for l in (1, 2)]
    mw1 = nc.declare_dram_parameter("mw1", [F, 2 * F], F32, isOutput=False)
    mb1 = nc.declare_dram_parameter("mb1", [F, 2], F32, isOutput=False)
    mw2 = nc.declare_dram_parameter("mw2", [P, 2, ncls], F32, isOutput=False)
    mb2 = nc.declare_dram_parameter("mb2", [P, ncls], F32, isOutput=False)
    out = nc.declare_dram_parameter("out", [g, ncls], F32, isOutput=True)

    # ---------------- internal DRAM ----------------
    m_local = nc.dram_tensor("m_local", [npc, F], BF16)
    m_full = nc.dram_tensor("m_full", [n, F], BF16, addr_space="Shared")
    pool_part = nc.dram_tensor("pool_part", [gblocks * P, F], F32)
    pool_red = nc.dram_tensor("pool_red", [gblocks * P, F], F32, addr_space="Shared")

    cc_groups = [list(range(ncores))]

    import contextlib
    es = contextlib.ExitStack()
    # ---------------- persistent SBUF state ----------------
    hT = es.enter_context(nc.sbuf_tensor("hT", [P, npc_pad], BF16))
    idx_t = es.enter_context(nc.sbuf_tensor("idx_t", [P, C * 8], I16))
    colloc_t = es.enter_context(nc.sbuf_tensor("colloc_t", [P, C], F32))
    wvals_t = es.enter_context(nc.sbuf_tensor("wvals_t", [P, C], F32))
    batchloc_t = es.enter_context(nc.sbuf_tensor("batchloc_t", [P, npairs], F32))
    cntinv_t = es.enter_context(nc.sbuf_tensor("cntinv_t", [P, gblocks], F32))
    iota_t = es.enter_context(nc.sbuf_tensor("iota_t", [P, P], F32))
    ident_t = es.enter_context(nc.sbuf_tensor("ident_t", [P, P], F32))
    ident_bf = es.enter_context(nc.sbuf_tensor("ident_bf", [P, P], BF16))
    wi_t = [es.enter_context(nc.sbuf_tensor(f"wi_t{l}", [F, F], BF16)) for l in range(2)]
    wr_t = [es.enter_context(nc.sbuf_tensor(f"wr_t{l}", [F, F], BF16)) for l in range(2)]
    b_t = [es.enter_context(nc.sbuf_tensor(f"b_t{l}", [F, 1], F32)) for l in range(2)]
    cc_sem = es.enter_context(nc.semaphore("cc_sem"))

    def m_pass(l, sm, psm):
        """m = h @ Wi -> m_local DRAM (node-major bf16)."""
        for ci in range(nw):
            s0, s1 = ci * P, (ci + 1) * P
            rows = min(npc - s0, P)
            mp = psm.tile([P, F], F32, tag="mps", name=f"mp{l}_{ci}", bufs=2)
            nc.tensor.matmul(out=mp[:], lhsT=hT[:, s0:s1], rhs=wi_t[l][:],
                             start=True, stop=True)
            ms = sm.tile([P, F], BF16, tag="ms", name=f"ms{l}_{ci}")
            nc.scalar.activation(out=ms[:], in_=mp[:], func=AF.Copy)
            nc.sync.dma_start(out=m_local[s0:s0 + rows, :], in_=ms[:rows, :])

    def conv_layer(l, tc, sm, msgp, ohp, psm):
        # batched gathers: one dma_gather per (superblock, group)
        msg_tiles = {}
        for ci_call, (sb, gi, c0, c1) in enumerate(calls):
            kb = c1 - c0
            if kb == 0:
                continue
            mt = msgp.tile([P, kbmax, F], BF16, tag=f"msg{gi}", name=f"msg{l}_{sb}_{gi}")
            nc.gpsimd.dma_gather(
                mt[:, :kb, :],
                m_full[gi * RANGE:min((gi + 1) * RANGE, n), :],
                idx_t[:, c0 * 8:c1 * 8],
                kb * P, kb * P, F,
                single_packet=False,
            )
            msg_tiles[ci_call] = mt
        # window loop: all matmuls for a window accumulate into one PSUM tile
        for w in range(nw):
            w0 = w * P
            e0 = min(npc, w0 + P)
            width = e0 - w0
            agg = psm.tile([F, P], F32, tag="agg", name=f"agg{l}_{w}", bufs=4)
            chunks = win_chunks[w]
            nmm = len(chunks) + 1
            # Wr path first (full 128 wide; hT pad cols are zero)
            nc.tensor.matmul(out=agg[:], lhsT=wr_t[l][:], rhs=hT[:, w0:w0 + P],
                             start=True, stop=(nmm == 1))
            for j, (ci_call, local) in enumerate(chunks):
                ch = calls[ci_call][2] + local
                oh = ohp.tile([P, P], BF16, tag="oh", name=f"oh{l}_{w}_{j}")
                nc.vector.tensor_scalar(
                    out=oh[:], in0=iota_t[:],
                    scalar1=colloc_t[:, ch:ch + 1], scalar2=wvals_t[:, ch:ch + 1],
                    op0=OP.is_equal, op1=OP.mult,
                )
                nc.tensor.matmul(out=agg[:],
                                 lhsT=msg_tiles[ci_call][:, local, :], rhs=oh[:],
                                 start=False, stop=(j == nmm - 2))
            nc.scalar.activation(out=hT[:, w0:e0], in_=agg[:, :width],
                                 func=AF.Relu, bias=b_t[l][:, 0:1], scale=1.0)

    # ================= phase A: loads + m1 =================
    with tile.TileContext(nc) as tc:
        with (
            tc.tile_pool(name="sm", bufs=4) as sm,
            tc.tile_pool(name="psm", bufs=6, space="PSUM") as psm,
        ):
            nc.sync.dma_start(out=iota_t[:], in_=iota_p[:])
            nc.sync.dma_start(out=ident_t[:], in_=ident_p[:])
            nc.vector.tensor_copy(ident_bf[:], ident_t[:])
            nc.sync.dma_start(out=idx_t[:], in_=idxs_p[:])
            nc.sync.dma_start(out=colloc_t[:], in_=colloc[:])
            nc.sync.dma_start(out=wvals_t[:], in_=wvals[:])
            nc.sync.dma_start(out=batchloc_t[:], in_=batchloc[:])
            nc.sync.dma_start(out=cntinv_t[:], in_=cntinv[:])
            for l in range(2):
                nc.sync.dma_start(out=wi_t[l][:], in_=wi[l][:])
                nc.sync.dma_start(out=wr_t[l][:], in_=wr[l][:])
                nc.sync.dma_start(out=b_t[l][:], in_=bb[l][:])
            nc.sync.dma_start(out=hT[:], in_=xT[:])
            m_pass(0, sm, psm)

    # ================= conv layers =================
    for l in range(2):
        nc.gpsimd.collective_compute(
            "AllGather", OP.bypass, replica_groups=cc_groups,
            ins=[m_local[:]], outs=[m_full[:]]).then_inc(cc_sem, 1)
        nc.gpsimd.wait_ge(cc_sem, l + 1)
        nc.sync.wait_ge(cc_sem, l + 1)

        with tile.TileContext(nc) as tc:
            with (
                tc.tile_pool(name="sm", bufs=4) as sm,
                tc.tile_pool(name="msgp", bufs=2) as msgp,
                tc.tile_pool(name="ohp", bufs=24) as ohp,
                tc.tile_pool(name="psm", bufs=6, space="PSUM") as psm,
            ):
                conv_layer(l, tc, sm, msgp, ohp, psm)
                if l == 0:
                    m_pass(1, sm, psm)

    # ================= pooling =================
    with tile.TileContext(nc) as tc:
        with (
            tc.tile_pool(name="sm", bufs=4) as sm,
            tc.tile_pool(name="ohp", bufs=8) as ohp,
            tc.tile_pool(name="psm", bufs=4, space="PSUM") as psm,
            tc.tile_pool(name="psPool", bufs=1, space="PSUM") as psPool,
        ):
            pool_ps = [psPool.tile([P, F], F32, tag=f"pp{b}", name=f"pool_ps{b}")
                       for b in range(gblocks)]
            cur_ci = -1
            nx = None
            for j, (ci, b, is_first, is_last) in enumerate(pair_flags):
                if ci != cur_ci:
                    s0 = ci * P
                    tp = psm.tile([P, F], BF16, tag="tp", name=f"tp{ci}", bufs=4)
                    nc.tensor.transpose(out=tp[:], in_=hT[:, s0:s0 + P],
                                        identity=ident_bf[:])
                    nx = sm.tile([P, F], BF16, tag="nx", name=f"nx{ci}")
                    nc.scalar.activation(out=nx[:], in_=tp[:], func=AF.Copy)
                    cur_ci = ci
                ohg = ohp.tile([P, P], BF16, tag="ohg", name=f"ohg{ci}_{b}")
                nc.vector.tensor_scalar(
                    out=ohg[:], in0=iota_t[:],
                    scalar1=batchloc_t[:, j:j + 1],
                    scalar2=None, op0=OP.is_equal)
                nc.tensor.matmul(out=pool_ps[b][:], lhsT=ohg[:], rhs=nx[:],
                                 start=is_first, stop=is_last)
            for b in range(gblocks):
                pps = sm.tile([P, F], F32, tag="pps", name=f"pps{b}")
                nc.scalar.activation(out=pps[:], in_=pool_ps[b][:], func=AF.Copy)
                nc.sync.dma_start(out=pool_part[b * P:(b + 1) * P, :], in_=pps[:])

    nc.gpsimd.collective_compute(
        "AllReduce", OP.add, replica_groups=cc_groups,
        ins=[pool_part[:]], outs=[pool_red[:]]).then_inc(cc_sem, 1)
    nc.sync.wait_ge(cc_sem, 3)

    # ================= mean + MLP head =================
    with tile.TileContext(nc) as tc:
        with (
            tc.tile_pool(name="sm", bufs=4) as sm,
            tc.tile_pool(name="one", bufs=1) as one,
            tc.tile_pool(name="psm", bufs=4, space="PSUM") as psm,
        ):
            meanT = one.tile([F, gblocks * P], F32)
            for b in range(gblocks):
                pr = sm.tile([P, F], F32, tag="pr", name=f"pr{b}")
                nc.sync.dma_start(out=pr[:], in_=pool_red[b * P:(b + 1) * P, :])
                mg = sm.tile([P, F], F32, tag="mg", name=f"mg{b}")
                nc.vector.tensor_scalar(out=mg[:], in0=pr[:],
                                        scalar1=cntinv_t[:, b:b + 1],
                                        scalar2=None, op0=OP.mult)
                mt = psm.tile([F, P], F32, tag="ps", name=f"mt{b}", bufs=2)
                nc.tensor.transpose(out=mt[:], in_=mg[:], identity=ident_t[:])
                nc.scalar.activation(out=meanT[:, b * P:(b + 1) * P], in_=mt[:],
                                     func=AF.Copy)
            mw1_t = one.tile([F, 2 * F], F32)
            nc.sync.dma_start(out=mw1_t[:], in_=mw1[:])
            mb1_t = one.tile([F, 2], F32)
            nc.sync.dma_start(out=mb1_t[:], in_=mb1[:])
            mw2_t = one.tile([P, 2, ncls], F32)
            nc.sync.dma_start(out=mw2_t[:], in_=mw2[:])
            mb2_t = one.tile([P, ncls], F32)
            nc.sync.dma_start(out=mb2_t[:], in_=mb2[:])
            hidT = one.tile([F, 2, gblocks * P], F32)
            for hc in range(2):
                hps = psm.tile([F, gblocks * P], F32, tag="hps", name=f"hps{hc}", bufs=2)
                nc.tensor.matmul(out=hps[:], lhsT=mw1_t[:, hc * F:(hc + 1) * F],
                                 rhs=meanT[:], start=True, stop=True)
                nc.scalar.activation(out=hidT[:, hc, :], in_=hps[:], func=AF.Relu,
                                     bias=mb1_t[:, hc:hc + 1], scale=1.0)
            p_out = min(P, g)
            outsb = one.tile([P, gblocks, ncls], F32)
            for gc in range(gblocks):
                ops_ = psm.tile([P, ncls], F32, tag="ps", name=f"ops{gc}", bufs=2)
                for hc in range(2):
                    nc.tensor.matmul(out=ops_[:], lhsT=hidT[:, hc, gc * P:(gc + 1) * P],
                                     rhs=mw2_t[:, hc, :],
                                     start=(hc == 0), stop=(hc == 1))
                nc.vector.tensor_tensor(out=outsb[:, gc, :], in0=ops_[:],
                                        in1=mb2_t[:], op=OP.add)
            nc.sync.dma_start(
                out=out.rearrange("(b p) c -> p b c", p=p_out),
                in_=outsb[:p_out, :, :])

    es.close()

    # SWDGE Q7 library load for InstDMAGatherAnt + ISA codegen
    import concourse.bacc as bacc
    bacc.Bacc.insert_library_loads(nc)
    mybir.codegen_inst_isa_subclasses(nc)
    return nc


def make_inputs(pre, x, Wi1, Wr1, b1, Wi2, Wr2, b2, mW1, mb1, mW2, mb2,
                n, ncores, g, ncls):
    """Build per-core in_maps."""
    npc = pre["npc"]
    npc_pad = pre["npc_pad"]
    iota = np.tile(np.arange(P, dtype=np.float32)[None, :], (P, 1))
    ident = np.eye(P, dtype=np.float32)
    x = np.asarray(x, np.float32)
    in_maps = []
    mb1w = np.ascontiguousarray(np.asarray(mb1, np.float32).reshape(2, P).T)
    mb2r = np.tile(np.asarray(mb2, np.float32).reshape(1, ncls), (P, 1))
    for c in range(ncores):
        xs = np.zeros((P, npc_pad), np.float32)
        xs[:, :npc] = x[c * npc:(c + 1) * npc, :].T
        m = dict(
            xT=to_bf16(xs),
            idxs=pre["cores"][c]["idxs"],
            colloc=pre["cores"][c]["colloc"],
            wvals=pre["cores"][c]["wvals"],
            batchloc=pre["cores"][c]["batchloc"],
            cntinv=pre["cntinv"],
            iota128=iota,
            ident128=ident,
            wi1=to_bf16(Wi1), wr1=to_bf16(Wr1),
            wi2=to_bf16(Wi2), wr2=to_bf16(Wr2),
            b1=np.asarray(b1, np.float32).reshape(P, 1),
            b2=np.asarray(b2, np.float32).reshape(P, 1),
            mw1=np.asarray(mW1, np.float32),
            mb1=mb1w,
            mw2=np.ascontiguousarray(
                np.asarray(mW2, np.float32).reshape(2, P, ncls).transpose(1, 0, 2)),
            mb2=mb2r,
        )
        in_maps.append(m)
    return in_maps


# ======================= entry point =======================
N_FULL = 100000
E_FULL = 640000
G_FULL = 512
NCLS_FULL = 2
NCORES = 8

_cache = {}


def kernel(x, edge_index, edge_attr, batch, Wi1, Wr1, b1, Wi2, Wr2, b2,
           mW1, mb1, mW2, mb2):
    install()
    x = np.asarray(x)
    edge_index = np.asarray(edge_index)
    edge_attr = np.asarray(edge_attr)
    batch = np.asarray(batch)
    n, f = x.shape
    g = G_FULL
    ncls = np.asarray(mW2).shape[1]

    pre = preprocess(edge_index, edge_attr, batch, n, NCORES, g)
    key = (n, g, ncls, pre["C"], pre["npairs"])
    if key not in _cache:
        nc = build_nc(pre, n, NCORES, g, ncls)
        _cache[key] = SpmdKernel(nc)
    k = _cache[key]
    in_maps = make_inputs(pre, x, Wi1, Wr1, b1, Wi2, Wr2, b2,
                          mW1, mb1, mW2, mb2, n, NCORES, g, ncls)
    ci, zz = k.put_inputs(in_maps)
    res = k.run_np(ci, zz)
    return np.ascontiguousarray(res[0]["out"].astype(np.float32))


# revision 22
# speedup vs baseline: 39.2279x; 1.1805x over previous
"""Self-contained Trainium2 Bass kernel for the ARMA GNN problem
(nn_ARMA_49297634623854).

kernel(**inputs) takes the FULL unsharded inputs (x, edge_index, edge_attr,
batch, Wi1, Wr1, b1, Wi2, Wr2, b2, mW1, mb1, mW2, mb2) as numpy arrays,
shards node-contiguously across 8 NeuronCores, runs an SPMD Bass kernel
(batched dma_gather edge gather + one-hot-matmul scatter fused in PSUM +
AllGather/AllReduce collectives), and returns the full [512, 2] float32
output.

Design notes:
- gcn_norm (deg/dinv) is computed on the host and baked into per-edge
  weights, so the device never computes degrees or rescales messages.
- Edge messages m = h @ Wi are AllGathered (bf16), then gathered per-edge
  with batched dma_gather (int16 indices -> 4 source-range groups of 25000
  rows each).
- Edges are grouped by (source-range group, 128-wide dest window); for each
  window all scatter matmuls plus the Wr-path matmul accumulate into one
  PSUM tile, and a single scalar-engine activation (relu + bias) writes the
  new hT window. No aggregation buffer, no vector adds.
"""

# ======================= walrus wait-splitting patches =======================
import concourse.mybir as mybir
import concourse.tile as tile
from concourse.vector_clock import ScopedClock, VectorClock

_nop_counter = [0]


def _make_wait_nop(engine, wait):
    _nop_counter[0] += 1
    return mybir.InstNoOp(
        name=f"SplitWait-{_nop_counter[0]}",
        engine=engine,
        ins=[],
        outs=[],
        sync_info=mybir.SyncInfo(on_wait=[wait], on_update=[]),
        bass_nofuse=True,
    )


def _split_multi_waits(insts):
    out = []
    n_split = 0
    for inst in insts:
        si = inst.sync_info
        if si is not None and len(si.on_wait) > 1:
            waits = list(si.on_wait)
            for w in waits[:-1]:
                out.append(_make_wait_nop(inst.engine, w))
            inst.sync_info = mybir.SyncInfo(
                on_wait=[waits[-1]], on_update=list(si.on_update)
            )
            n_split += 1
        out.append(inst)
    return out, n_split


_orig_lower = tile.TileContext._lower_ordered_insts


def _patched_lower(self, postordered_blocks):
    total = 0
    for bbname in list(postordered_blocks.keys()):
        newlist, n = _split_multi_waits(postordered_blocks[bbname])
        postordered_blocks[bbname] = newlist
        total += n
    return _orig_lower(self, postordered_blocks)


def _patched_drain_and_barrier(self, tick_clock, wait_clock):
    gc = tick_clock.global_clock
    nprocs = len(gc)
    for p in range(nprocs):
        t = gc[p]
        if t <= 0:
            continue
        vec = [0] * nprocs
        vec[p] = t
        nop_inst = self.nc.sync.nop(nofuse=True)
        wait_clock.add_sem_waits(nop_inst.ins, ScopedClock({None: VectorClock(vec)}))
    self.nc.sync.drain()
    self.nc.all_engine_barrier()
    assert self.sems is not None
    popped = self.nc._tile_sem_poison_stack.pop()
    assert popped is self._sem_poison
    self.nc.clear_and_free_semaphores(list(self.sems.allocated().values()))
    self.nc.all_engine_barrier()


def install():
    tile.TileContext._lower_ordered_insts = _patched_lower
    tile.TileContext._drain_and_barrier = _patched_drain_and_barrier


# ======================= SPMD runner =======================
import time

import jax
import numpy as np
from jax.sharding import Mesh, NamedSharding, PartitionSpec
from jax.experimental.shard_map import shard_map

import concourse.bass as bass
import concourse.mybir as mybir
from concourse import bass2jax
from concourse.bass2jax import (
    _bass_exec_p,
    fast_dispatch_compile,
    install_neuronx_cc_hook,
    partition_id_tensor,
)


class SpmdKernel:
    def __init__(self, nc: bass.Bass, n_cores: int = 8):
        install_neuronx_cc_hook()
        self.nc = nc
        self.n_cores = n_cores
        in_names: list[str] = []
        out_names: list[str] = []
        out_avals: list[jax.core.ShapedArray] = []
        partition_name = (
            nc.partition_id_tensor.name if nc.partition_id_tensor else None
        )
        for alloc in nc.m.functions[0].allocations:
            if not isinstance(alloc, mybir.MemoryLocationSet):
                continue
            name = alloc.memorylocations[0].name
            if alloc.kind == "ExternalInput":
                if name != partition_name:
                    in_names.append(name)
            elif alloc.kind == "ExternalOutput":
                shape = tuple(alloc.tensor_shape)
                dtype = mybir.dt.np(alloc.dtype)
                out_names.append(name)
                out_avals.append(jax.core.ShapedArray(shape, dtype))
        self.n_params = len(in_names)
        self.out_names = out_names
        self.out_avals = out_avals
        self.in_names = in_names[:]
        all_in_names = in_names + out_names
        if partition_name is not None:
            all_in_names.append(partition_name)

        def _body(*args):
            operands = list(args)
            if partition_name is not None:
                operands.append(partition_id_tensor())
            outs = _bass_exec_p.bind(
                *operands,
                out_avals=tuple(out_avals),
                in_names=tuple(all_in_names),
                out_names=tuple(out_names),
                lowering_input_output_aliases=(),
                sim_require_finite=True,
                sim_require_nnan=True,
                nc=nc,
            )
            return tuple(outs)

        devices = jax.devices()[: n_cores]
        assert len(devices) == n_cores
        self.mesh = Mesh(np.asarray(devices), ("core",))
        n_out = len(out_names)
        in_specs = (PartitionSpec("core"),) * (self.n_params + n_out)
        out_specs = (PartitionSpec("core"),) * n_out
        self._sharded = shard_map(
            _body,
            mesh=self.mesh,
            in_specs=in_specs,
            out_specs=out_specs,
            check_rep=False,
        )
        self.fn = jax.jit(self._sharded, keep_unused=True)
        self._compiled = None
        self.sharding = NamedSharding(self.mesh, PartitionSpec("core"))

    def compile_fast(self, concat_in, zeros):
        """AOT compile with fast dispatch (no effects)."""
        self._compiled = fast_dispatch_compile(
            lambda: jax.jit(self._sharded, keep_unused=True)
            .lower(*concat_in, *zeros)
            .compile()
        )
        return self._compiled

    def put_inputs(self, in_maps: list[dict[str, np.ndarray]]):
        """in_maps: one dict per core. Returns list of device arrays (concat
        along axis 0) in in_names order, plus zero output buffers."""
        concat_in = []
        for name in self.in_names:
            arrs = [np.asarray(in_maps[c][name]) for c in range(self.n_cores)]
            concat_in.append(
                jax.device_put(np.concatenate(arrs, axis=0), self.sharding)
            )
        zeros = []
        for av in self.out_avals:
            z = np.zeros((self.n_cores * av.shape[0], *av.shape[1:]), av.dtype)
            zeros.append(jax.device_put(z, self.sharding))
        return concat_in, zeros

    def __call__(self, concat_in, zeros):
        f = self._compiled or self.fn
        outs = f(*concat_in, *zeros)
        return outs

    def run_np(self, concat_in, zeros):
        f = self._compiled or self.fn
        outs = f(*concat_in, *zeros)
        res = []
        for c in range(self.n_cores):
            res.append(
                {
                    name: np.asarray(outs[i]).reshape(
                        self.n_cores, *self.out_avals[i].shape
                    )[c]
                    for i, name in enumerate(self.out_names)
                }
            )
        return res

    def time_it(self, concat_in, zeros, reps=20, warmup=3):
        f = self._compiled or self.fn
        for _ in range(warmup):
            jax.block_until_ready(f(*concat_in, *zeros))
        ts = []
        for _ in range(reps):
            t0 = time.perf_counter()
            jax.block_until_ready(f(*concat_in, *zeros))
            ts.append(time.perf_counter() - t0)
        return min(ts), sorted(ts)[len(ts) // 2]


# ======================= GNN kernel builder =======================
import concourse.bass as bass
import concourse.mybir as mybir
import concourse.tile as tile

F32 = mybir.dt.float32
BF16 = mybir.dt.bfloat16
I16 = mybir.dt.int16
AF = mybir.ActivationFunctionType
OP = mybir.AluOpType
P = 128
RANGE = 25000  # int16-safe dma_gather source range
NSB = 8        # gather superblocks per layer per group


def wrap128(arr):
    """[C*128,...] -> [128, C] layout: out[p, c] = arr[c*128+p]."""
    C = arr.shape[0] // 128
    return np.ascontiguousarray(arr.reshape(C, 128).T)


def to_bf16(a):
    import jax.numpy as jnp
    return np.asarray(jnp.asarray(np.asarray(a, np.float32), dtype=jnp.bfloat16))


def preprocess(edge_index, edge_attr, batch, n, ncores, g):
    """Host-side: gcn_norm, per-core edge->chunk tables, pooling tables.

    Chunks are 128 edge slots grouped by (source-range group, dest window).
    Chunk ordering: for sb: for group: for window in sb: chunks -- so each
    (sb, group) is one contiguous dma_gather call span.
    """
    npc = n // ncores
    nw = (npc + 127) // 128
    npc_pad = nw * P
    ngroups = (n + RANGE - 1) // RANGE
    gblocks = (g + P - 1) // P

    row = np.asarray(edge_index[0], np.int64)
    col = np.asarray(edge_index[1], np.int64)
    w_attr = np.asarray(edge_attr, np.float32).reshape(-1)
    batch = np.asarray(batch, np.int64)

    deg = np.bincount(col, weights=w_attr.astype(np.float64), minlength=n)
    dinv = np.where(deg > 0, 1.0 / np.sqrt(np.maximum(deg, 1e-12)), 0.0)
    norm = (dinv[row] * w_attr * dinv[col]).astype(np.float32)

    per_core = []
    cnts = np.zeros((ncores, ngroups, nw), np.int64)
    for c in range(ncores):
        m = (col // npc) == c
        r_c = row[m]
        d_c = col[m] - c * npc
        n_c = norm[m]
        g_c = r_c // RANGE
        w_c = d_c // P
        o = np.lexsort((r_c, w_c, g_c))
        r_c, d_c, n_c, g_c, w_c = r_c[o], d_c[o], n_c[o], g_c[o], w_c[o]
        per_core.append((r_c, d_c, n_c, g_c, w_c))
        for gi in range(ngroups):
            cnts[c, gi] = np.bincount(w_c[g_c == gi], minlength=nw)
    nch = ((cnts.max(axis=0) + P - 1) // P).astype(np.int64)  # [ngroups, nw]

    # window -> superblock (balanced, monotone)
    wsb = (np.arange(nw) * NSB) // nw
    chunk_of = {}
    calls = []  # (sb, g, c0, c1)
    C = 0
    for sb in range(NSB):
        ws = [w for w in range(nw) if wsb[w] == sb]
        for gi in range(ngroups):
            c0 = C
            for w in ws:
                chunk_of[(gi, w)] = C
                C += int(nch[gi, w])
            calls.append((sb, gi, c0, C))
    kbmax = max(c1 - c0 for (_, _, c0, c1) in calls)

    # per-window chunk list: (call_index, local_col) per chunk
    win_chunks = []  # [nw] -> list of (call_idx, local_col)
    call_idx_of = {}
    for i, (sb, gi, c0, c1) in enumerate(calls):
        call_idx_of[(sb, gi)] = i
    for w in range(nw):
        sb = int(wsb[w])
        lst = []
        for gi in range(ngroups):
            k = int(nch[gi, w])
            if k == 0:
                continue
            ci = call_idx_of[(sb, gi)]
            c0 = calls[ci][2]
            base = chunk_of[(gi, w)] - c0
            for j in range(k):
                lst.append((ci, base + j))
        win_chunks.append(lst)

    cores = []
    for c in range(ncores):
        r_c, d_c, n_c, g_c, w_c = per_core[c]
        idx16 = np.zeros(C * P, np.int16)
        colloc = np.zeros(C * P, np.float32)
        wvals = np.zeros(C * P, np.float32)
        key = g_c * nw + w_c
        uniq, starts = np.unique(key, return_index=True)
        ends = np.r_[starts[1:], len(key)]
        for u, s0, s1 in zip(uniq, starts, ends):
            gi, w = divmod(int(u), nw)
            base = chunk_of[(gi, w)] * P
            cnt = int(s1 - s0)
            sl = slice(base, base + cnt)
            idx16[sl] = (r_c[s0:s1] - gi * RANGE).astype(np.int16)
            colloc[sl] = (d_c[s0:s1] - w * P).astype(np.float32)
            wvals[sl] = n_c[s0:s1]
        idx_t = np.tile(np.ascontiguousarray(idx16.reshape(C * 8, 16).T), (8, 1))
        cores.append(
            dict(
                idxs=np.ascontiguousarray(idx_t),
                colloc=wrap128(colloc),
                wvals=wrap128(wvals),
            )
        )

    # pooling: per-core local graph index (slab spans <= 128 graphs)
    for c in range(ncores):
        bl = batch[c * npc:(c + 1) * npc]
        g0, g1 = int(bl[0]), int(bl[-1])
        assert g1 - g0 + 1 <= P, f"core {c} graph span {g1 - g0 + 1} > 128"
        blloc = np.full(npc_pad, -1.0, np.float32)
        blloc[:npc] = (bl - g0).astype(np.float32)
        cores[c]["batchloc"] = wrap128(blloc)  # [P, nw]
        cores[c]["growidx"] = (g0 + np.arange(P, dtype=np.int32)).reshape(P, 1)

    cnt = np.bincount(batch, minlength=g).astype(np.float64)
    cnt_inv = (1.0 / np.maximum(cnt, 1.0)).astype(np.float32)  # [g]
    cntinv_t = np.ascontiguousarray(cnt_inv.reshape(gblocks, P).T)  # [P, gblocks]

    return dict(
        C=C, calls=calls, win_chunks=win_chunks, kbmax=kbmax, nw=nw,
        npc=npc, npc_pad=npc_pad, ngroups=ngroups, gblocks=gblocks,
        cores=cores, cntinv=cntinv_t, wsb=[int(x) for x in wsb],
    )


def build_nc(pre, n, ncores, g, ncls):
    F = 128
    C = pre["C"]
    nw = pre["nw"]
    npc = pre["npc"]
    npc_pad = pre["npc_pad"]
    ngroups = pre["ngroups"]
    gblocks = pre["gblocks"]
    calls = pre["calls"]
    win_chunks = pre["win_chunks"]
    kbmax = pre["kbmax"]
    # descriptor-ring capacity check: ~255 descs/engine-ring with 32KB scratch
    assert kbmax * P // 16 + 1 <= 248, f"gather call too big: kbmax={kbmax}"

    nc = bass.Bass(dynamic_dma_scratch_size=32768)

    # ---------------- parameters ----------------
    xT = nc.declare_dram_parameter("xT", [P, npc_pad], BF16, isOutput=False)
    idxs_p = nc.declare_dram_parameter("idxs", [P, C * 8], I16, isOutput=False)
    colloc = nc.declare_dram_parameter("colloc", [P, C], F32, isOutput=False)
    wvals = nc.declare_dram_parameter("wvals", [P, C], F32, isOutput=False)
    batchloc = nc.declare_dram_parameter("batchloc", [P, nw], F32, isOutput=False)
    growidx = nc.declare_dram_parameter("growidx", [P, 1], mybir.dt.int32, isOutput=False)
    cntinv = nc.declare_dram_parameter("cntinv", [P, gblocks], F32, isOutput=False)
    iota_p = nc.declare_dram_parameter("iota128", [P, P], F32, isOutput=False)
    ident_p = nc.declare_dram_parameter("ident128", [P, P], F32, isOutput=False)
    wi = [nc.declare_dram_parameter(f"wi{l}", [F, F], BF16, isOutput=False) for l in (1, 2)]
    wr = [nc.declare_dram_parameter(f"wr{l}", [F, F], BF16, isOutput=False) for l in (1, 2)]
    bb = [nc.declare_dram_parameter(f"b{l}", [F, 1], F32, isOutput=False) for l in (1, 2)]
    mw1 = nc.declare_dram_parameter("mw1", [F, 2 * F], F32, isOutput=False)
    mb1 = nc.declare_dram_parameter("mb1", [F, 2], F32, isOutput=False)
    mw2 = nc.declare_dram_parameter("mw2", [P, 2, ncls], F32, isOutput=False)
    mb2 = nc.declare_dram_parameter("mb2", [P, ncls], F32, isOutput=False)
    out = nc.declare_dram_parameter("out", [g, ncls], F32, isOutput=True)

    # ---------------- internal DRAM ----------------
    m_local = nc.dram_tensor("m_local", [npc, F], BF16)
    m_full = nc.dram_tensor("m_full", [n, F], BF16, addr_space="Shared")
    pool_part = nc.dram_tensor("pool_part", [gblocks * P, F], F32)
    pool_red = nc.dram_tensor("pool_red", [gblocks * P, F], F32, addr_space="Shared")

    cc_groups = [list(range(ncores))]

    import contextlib
    es = contextlib.ExitStack()
    # ---------------- persistent SBUF state ----------------
    hT = es.enter_context(nc.sbuf_tensor("hT", [P, npc_pad], BF16))
    idx_t = es.enter_context(nc.sbuf_tensor("idx_t", [P, C * 8], I16))
    colloc_t = es.enter_context(nc.sbuf_tensor("colloc_t", [P, C], F32))
    wvals_t = es.enter_context(nc.sbuf_tensor("wvals_t", [P, C], F32))
    batchloc_t = es.enter_context(nc.sbuf_tensor("batchloc_t", [P, nw], F32))
    growidx_t = es.enter_context(nc.sbuf_tensor("growidx_t", [P, 1], mybir.dt.int32))
    cntinv_t = es.enter_context(nc.sbuf_tensor("cntinv_t", [P, gblocks], F32))
    iota_t = es.enter_context(nc.sbuf_tensor("iota_t", [P, P], F32))
    ident_t = es.enter_context(nc.sbuf_tensor("ident_t", [P, P], F32))
    ident_bf = es.enter_context(nc.sbuf_tensor("ident_bf", [P, P], BF16))
    wi_t = [es.enter_context(nc.sbuf_tensor(f"wi_t{l}", [F, F], BF16)) for l in range(2)]
    wr_t = [es.enter_context(nc.sbuf_tensor(f"wr_t{l}", [F, F], BF16)) for l in range(2)]
    b_t = [es.enter_context(nc.sbuf_tensor(f"b_t{l}", [F, 1], F32)) for l in range(2)]
    cc_sem = es.enter_context(nc.semaphore("cc_sem"))

    def m_pass(l, sm, psm):
        """m = h @ Wi -> m_local DRAM (node-major bf16)."""
        for ci in range(nw):
            s0, s1 = ci * P, (ci + 1) * P
            rows = min(npc - s0, P)
            mp = psm.tile([P, F], F32, tag="mps", name=f"mp{l}_{ci}", bufs=2)
            nc.tensor.matmul(out=mp[:], lhsT=hT[:, s0:s1], rhs=wi_t[l][:],
                             start=True, stop=True)
            ms = sm.tile([P, F], BF16, tag="ms", name=f"ms{l}_{ci}")
            nc.scalar.activation(out=ms[:], in_=mp[:], func=AF.Copy)
            nc.sync.dma_start(out=m_local[s0:s0 + rows, :], in_=ms[:rows, :])

    _reg_cache = {}

    def reg_of(val):
        if val not in _reg_cache:
            _reg_cache[val] = nc.gpsimd.to_reg(val)
        return _reg_cache[val]

    def conv_layer(l, tc, sm, msgp, ohp, psm, agg_bufs):
        # batched gathers + batched one-hot gen: one of each per (sb, group)
        msg_tiles = {}
        oh_tiles = {}
        for ci_call, (sb, gi, c0, c1) in enumerate(calls):
            kb = c1 - c0
            if kb == 0:
                continue
            mt = msgp.tile([P, kbmax, F], BF16, tag=f"msg{gi}", name=f"msg{l}_{sb}_{gi}")
            nc.gpsimd.dma_gather(
                mt[:, :kb, :],
                m_full[gi * RANGE:min((gi + 1) * RANGE, n), :],
                idx_t[:, c0 * 8:c1 * 8],
                kb * P, reg_of(kb * P), F,
                single_packet=False,
            )
            msg_tiles[ci_call] = mt
            ohb = ohp.tile([P, kbmax, P], BF16, tag=f"ohb{gi}", name=f"ohb{l}_{sb}_{gi}")
            iota_bc = iota_t[:].rearrange("p (one j) -> p one j", one=1).to_broadcast([P, kb, P])
            colloc_bc = colloc_t[:, c0:c1].rearrange("p (c one) -> p c one", one=1).to_broadcast([P, kb, P])
            wvals_bc = wvals_t[:, c0:c1].rearrange("p (c one) -> p c one", one=1).to_broadcast([P, kb, P])
            nc.vector.tensor_tensor(out=ohb[:, :kb, :], in0=iota_bc, in1=colloc_bc,
                                    op=OP.is_equal)
            nc.vector.tensor_tensor(out=ohb[:, :kb, :], in0=ohb[:, :kb, :],
                                    in1=wvals_bc, op=OP.mult)
            oh_tiles[ci_call] = ohb
        # window loop: all matmuls for a window accumulate into one PSUM tile
        for w in range(nw):
            w0 = w * P
            e0 = min(npc, w0 + P)
            width = e0 - w0
            agg = psm.tile([F, P], F32, tag="agg", name=f"agg{l}_{w}", bufs=agg_bufs)
            chunks = win_chunks[w]
            nmm = len(chunks) + 1
            # Wr path first (full 128 wide; hT pad cols are zero)
            nc.tensor.matmul(out=agg[:], lhsT=wr_t[l][:], rhs=hT[:, w0:w0 + P],
                             start=True, stop=(nmm == 1))
            for j, (ci_call, local) in enumerate(chunks):
                nc.tensor.matmul(out=agg[:],
                                 lhsT=msg_tiles[ci_call][:, local, :],
                                 rhs=oh_tiles[ci_call][:, local, :],
                                 start=False, stop=(j == nmm - 2))
            nc.scalar.activation(out=hT[:, w0:e0], in_=agg[:, :width],
                                 func=AF.Relu, bias=b_t[l][:, 0:1], scale=1.0)

    # ================= phase A: loads + m1 =================
    with tile.TileContext(nc) as tc:
        with (
            tc.tile_pool(name="sm", bufs=4) as sm,
            tc.tile_pool(name="psm", bufs=6, space="PSUM") as psm,
        ):
            nc.sync.dma_start(out=iota_t[:], in_=iota_p[:])
            nc.sync.dma_start(out=ident_t[:], in_=ident_p[:])
            nc.vector.tensor_copy(ident_bf[:], ident_t[:])
            nc.sync.dma_start(out=idx_t[:], in_=idxs_p[:])
            nc.sync.dma_start(out=colloc_t[:], in_=colloc[:])
            nc.sync.dma_start(out=wvals_t[:], in_=wvals[:])
            nc.sync.dma_start(out=batchloc_t[:], in_=batchloc[:])
            nc.sync.dma_start(out=growidx_t[:], in_=growidx[:])
            nc.sync.dma_start(out=cntinv_t[:], in_=cntinv[:])
            for l in range(2):
                nc.sync.dma_start(out=wi_t[l][:], in_=wi[l][:])
                nc.sync.dma_start(out=wr_t[l][:], in_=wr[l][:])
                nc.sync.dma_start(out=b_t[l][:], in_=bb[l][:])
            nc.sync.dma_start(out=hT[:], in_=xT[:])
            m_pass(0, sm, psm)

    # ================= conv layers (+ pooling fused into conv1) =================
    for l in range(2):
        nc.gpsimd.collective_compute(
            "AllGather", OP.bypass, replica_groups=cc_groups,
            ins=[m_local[:]], outs=[m_full[:]]).then_inc(cc_sem, 1)
        nc.gpsimd.wait_ge(cc_sem, l + 1)
        nc.sync.wait_ge(cc_sem, l + 1)

        with tile.TileContext(nc) as tc:
            with (
                tc.tile_pool(name="sm", bufs=4) as sm,
                tc.tile_pool(name="msgp", bufs=2) as msgp,
                tc.tile_pool(name="ohp", bufs=2) as ohp,
                tc.tile_pool(name="psm", bufs=6, space="PSUM") as psm,
                tc.tile_pool(name="psPool", bufs=1, space="PSUM") as psPool,
            ):
                conv_layer(l, tc, sm, msgp, ohp, psm, agg_bufs=4 if l == 0 else 3)
                if l == 0:
                    m_pass(1, sm, psm)
                else:
                    # ---- pooling: per-core local-graph sums + indirect scatter
                    zt = sm.tile([P, F], F32, tag="zt", name="zt")
                    nc.vector.memset(zt[:], 0.0)
                    for b in range(gblocks):
                        nc.sync.dma_start(out=pool_part[b * P:(b + 1) * P, :],
                                          in_=zt[:])
                    ohgb = ohp.tile([P, nw, P], BF16, tag="ohgb", name="ohgb", bufs=1)
                    iota_bc = iota_t[:].rearrange("p (one j) -> p one j", one=1).to_broadcast([P, nw, P])
                    bl_bc = batchloc_t[:].rearrange("p (c one) -> p c one", one=1).to_broadcast([P, nw, P])
                    nc.vector.tensor_tensor(out=ohgb[:], in0=iota_bc, in1=bl_bc,
                                            op=OP.is_equal)
                    pool_acc = psPool.tile([P, F], F32, tag="pacc", name="pool_acc")
                    for ci in range(nw):
                        s0 = ci * P
                        tp = psm.tile([P, F], BF16, tag="tp", name=f"tp{ci}", bufs=3)
                        nc.tensor.transpose(out=tp[:], in_=hT[:, s0:s0 + P],
                                            identity=ident_bf[:])
                        nx = sm.tile([P, F], BF16, tag="nx", name=f"nx{ci}")
                        nc.scalar.activation(out=nx[:], in_=tp[:], func=AF.Copy)
                        nc.tensor.matmul(out=pool_acc[:], lhsT=ohgb[:, ci, :],
                                         rhs=nx[:],
                                         start=(ci == 0), stop=(ci == nw - 1))
                    pool_loc = sm.tile([P, F], F32, tag="ploc", name="pool_loc")
                    nc.scalar.activation(out=pool_loc[:], in_=pool_acc[:],
                                         func=AF.Copy)
                    nc.gpsimd.indirect_dma_start(
                        out=pool_part[:],
                        out_offset=bass.IndirectOffsetOnAxis(
                            ap=growidx_t[:, 0:1], axis=0),
                        in_=pool_loc[:], in_offset=None,
                        bounds_check=g - 1, oob_is_err=False)

    nc.gpsimd.collective_compute(
        "AllReduce", OP.add, replica_groups=cc_groups,
        ins=[pool_part[:]], outs=[pool_red[:]]).then_inc(cc_sem, 1)
    nc.sync.wait_ge(cc_sem, 3)

    # ================= mean + MLP head =================
    with tile.TileContext(nc) as tc:
        with (
            tc.tile_pool(name="sm", bufs=4) as sm,
            tc.tile_pool(name="one", bufs=1) as one,
            tc.tile_pool(name="psm", bufs=4, space="PSUM") as psm,
        ):
            meanT = one.tile([F, gblocks * P], F32)
            for b in range(gblocks):
                pr = sm.tile([P, F], F32, tag="pr", name=f"pr{b}")
                nc.sync.dma_start(out=pr[:], in_=pool_red[b * P:(b + 1) * P, :])
                mg = sm.tile([P, F], F32, tag="mg", name=f"mg{b}")
                nc.vector.tensor_scalar(out=mg[:], in0=pr[:],
                                        scalar1=cntinv_t[:, b:b + 1],
                                        scalar2=None, op0=OP.mult)
                mt = psm.tile([F, P], F32, tag="ps", name=f"mt{b}", bufs=2)
                nc.tensor.transpose(out=mt[:], in_=mg[:], identity=ident_t[:])
                nc.scalar.activation(out=meanT[:, b * P:(b + 1) * P], in_=mt[:],
                                     func=AF.Copy)
            mw1_t = one.tile([F, 2 * F], F32)
            nc.sync.dma_start(out=mw1_t[:], in_=mw1[:])
            mb1_t = one.tile([F, 2], F32)
            nc.sync.dma_start(out=mb1_t[:], in_=mb1[:])
            mw2_t = one.tile([P, 2, ncls], F32)
            nc.sync.dma_start(out=mw2_t[:], in_=mw2[:])
            mb2_t = one.tile([P, ncls], F32)
            nc.sync.dma_start(out=mb2_t[:], in_=mb2[:])
            hidT = one.tile([F, 2, gblocks * P], F32)
            for hc in range(2):
                hps = psm.tile([F, gblocks * P], F32, tag="hps", name=f"hps{hc}", bufs=2)
                nc.tensor.matmul(out=hps[:], lhsT=mw1_t[:, hc * F:(hc + 1) * F],
                                 rhs=meanT[:], start=True, stop=True)
                nc.scalar.activation(out=hidT[:, hc, :], in_=hps[:], func=AF.Relu,
                                     bias=mb1_t[:, hc:hc + 1], scale=1.0)
            p_out = min(P, g)
            outsb = one.tile([P, gblocks, ncls], F32)
            for gc in range(gblocks):
                ops_ = psm.tile([P, ncls], F32, tag="ps", name=f"ops{gc}", bufs=2)
                for hc in range(2):
                    nc.tensor.matmul(out=ops_[:], lhsT=hidT[:, hc, gc * P:(gc + 1) * P],
                                     rhs=mw2_t[:, hc, :],
                                     start=(hc == 0), stop=(hc == 1))
                nc.vector.tensor_tensor(out=outsb[:, gc, :], in0=ops_[:],
                                        in1=mb2_t[:], op=OP.add)
            nc.sync.dma_start(
                out=out.rearrange("(b p) c -> p b c", p=p_out),
                in_=outsb[:p_out, :, :])

    es.close()

    # SWDGE Q7 library load for InstDMAGatherAnt + ISA codegen
    import concourse.bacc as bacc
    bacc.Bacc.insert_library_loads(nc)
    mybir.codegen_inst_isa_subclasses(nc)
    return nc


def make_inputs(pre, x, Wi1, Wr1, b1, Wi2, Wr2, b2, mW1, mb1, mW2, mb2,
                n, ncores, g, ncls):
    """Build per-core in_maps."""
    npc = pre["npc"]
    npc_pad = pre["npc_pad"]
    iota = np.tile(np.arange(P, dtype=np.float32)[None, :], (P, 1))
    ident = np.eye(P, dtype=np.float32)
    x = np.asarray(x, np.float32)
    in_maps = []
    mb1w = np.ascontiguousarray(np.asarray(mb1, np.float32).reshape(2, P).T)
    mb2r = np.tile(np.asarray(mb2, np.float32).reshape(1, ncls), (P, 1))
    for c in range(ncores):
        xs = np.zeros((P, npc_pad), np.float32)
        xs[:, :npc] = x[c * npc:(c + 1) * npc, :].T
        m = dict(
            xT=to_bf16(xs),
            idxs=pre["cores"][c]["idxs"],
            colloc=pre["cores"][c]["colloc"],
            wvals=pre["cores"][c]["wvals"],
            batchloc=pre["cores"][c]["batchloc"],
            growidx=pre["cores"][c]["growidx"],
            cntinv=pre["cntinv"],
            iota128=iota,
            ident128=ident,
            wi1=to_bf16(Wi1), wr1=to_bf16(Wr1),
            wi2=to_bf16(Wi2), wr2=to_bf16(Wr2),
            b1=np.asarray(b1, np.float32).reshape(P, 1),
            b2=np.asarray(b2, np.float32).reshape(P, 1),
            mw1=np.asarray(mW1, np.float32),
            mb1=mb1w,
            mw2=np.ascontiguousarray(
                np.asarray(mW2, np.float32).reshape(2, P, ncls).transpose(1, 0, 2)),
            mb2=mb2r,
        )
        in_maps.append(m)
    return in_maps


# ======================= entry point =======================
N_FULL = 100000
E_FULL = 640000
G_FULL = 512
NCLS_FULL = 2
NCORES = 8

_cache = {}


def kernel(x, edge_index, edge_attr, batch, Wi1, Wr1, b1, Wi2, Wr2, b2,
           mW1, mb1, mW2, mb2):
    install()
    x = np.asarray(x)
    edge_index = np.asarray(edge_index)
    edge_attr = np.asarray(edge_attr)
    batch = np.asarray(batch)
    n, f = x.shape
    g = G_FULL
    ncls = np.asarray(mW2).shape[1]

    pre = preprocess(edge_index, edge_attr, batch, n, NCORES, g)
    key = (n, g, ncls, pre["C"])
    if key not in _cache:
        nc = build_nc(pre, n, NCORES, g, ncls)
        _cache[key] = SpmdKernel(nc)
    k = _cache[key]
    in_maps = make_inputs(pre, x, Wi1, Wr1, b1, Wi2, Wr2, b2,
                          mW1, mb1, mW2, mb2, n, NCORES, g, ncls)
    ci, zz = k.put_inputs(in_maps)
    res = k.run_np(ci, zz)
    return np.ascontiguousarray(res[0]["out"].astype(np.float32))
